# revision 34
# baseline (speedup 1.0000x reference)
"""ChildSum TreeLSTM (relational) — Trainium2 Bass kernel, 8 NeuronCores.

Strategy (data-parallel over batch, per sharding hint):
  - 16 trees are split over 8 cores, 2 whole trees per core.
  - Inside each core, nodes are relabeled level-by-level (sorted by tree
    height) so each bottom-up level occupies a contiguous row range of a
    padded node space.  All per-level gathers/scatters become small
    matmuls against host-built 0/1 incidence matrices (exact in fp).
  - bf16 everywhere on the matmul path (tolerance 2e-2; measured err
    ~3e-3).  bf16 streams 1 row/cycle at ANY width, so incidence blocks
    are packed to exact/32-aligned widths instead of 256-col padding.
  - Per-level gate math runs on a 32-aligned partition window of the
    target ptile (engine APs may start at partitions 0/32/64/96), and
    commits into the h/c state are masked with copy_predicated.
  - u/i/o PSUM tiles are pre-seeded with the (hoisted) input projection
    via identity matmuls that land in the previous level's tail; the
    recurrent matmuls accumulate on top.
  - Per-level emission is stale-first so the in-order PE queue drains
    next-level gather work during each level's activation/commit tail.
  - Readout is incremental: a node ptile is pooled as soon as the last
    level writing it commits (+4 presence-offset trick so absent slots
    never win the max).
  - Embedding/rel rows are gathered and transposed on the host (pure
    input indexing, part of the sharding step) and packed into the
    per-core constant block; LSTM weights are replicated to every core.

The SPMD program is identical on all cores; per-core behavior differs
only through input data (index vectors + incidence matrices).  Level
sizes are padded to the max across cores.
"""

import os
import numpy as np

P = 128
H = 256
HT = H // P          # h-state partition tiles
G3 = 3 * H           # packed i|o|u width (768)
N_CORES = 8
POOL_OFS = 4.0       # presence offset for incremental max-pooling


# ----------------------------------------------------------------------------
# Host-side plan builder
# ----------------------------------------------------------------------------

def _ceil_to(x, m):
    return (x + m - 1) // m * m


def _split_chunks(row0, cnt):
    """Split a row range into pieces that don't straddle 128-partition tiles."""
    out = []
    r, remaining = row0, cnt
    while remaining > 0:
        take = min(P - (r % P), remaining)
        out.append((r, take))
        r += take
        remaining -= take
    return out


def build_plan(xs, rels, child_idx, parent_idx, node_height, n_levels,
               n_cores=N_CORES):
    xs = np.asarray(xs)
    rels = np.asarray(rels)
    B, S = xs.shape
    tpc = B // n_cores
    heights = np.asarray(node_height).reshape(B, S)
    ci = np.asarray(child_idx)
    pi = np.asarray(parent_idx)
    NL = min(int(heights.max()) + 1, int(n_levels))

    edges_by_parent = {}
    for c, p in zip(ci.tolist(), pi.tolist()):
        edges_by_parent.setdefault(p, []).append(c)

    core_nodes, core_edges = [], []
    for core in range(n_cores):
        nl = [[] for _ in range(NL)]
        el = [[] for _ in range(NL)]
        for t in range(tpc):
            b = core * tpc + t
            for s in range(S):
                h = int(heights[b, s])
                if h < NL:
                    nl[h].append((t, s))
        for lv in range(1, NL):
            for (t, s) in nl[lv]:
                pg = (core * tpc + t) * S + s
                for cg in edges_by_parent.get(pg, []):
                    el[lv].append((cg, pg))
        core_nodes.append(nl)
        core_edges.append(el)

    n_hat = [max(len(core_nodes[c][lv]) for c in range(n_cores)) for lv in range(NL)]
    e_hat = [max(len(core_edges[c][lv]) for c in range(n_cores)) for lv in range(NL)]
    n_off = [0]
    for v in n_hat:
        n_off.append(n_off[-1] + v)
    e_off = [0]
    for v in e_hat:
        e_off.append(e_off[-1] + v)
    Npad = max(P, _ceil_to(n_off[-1], P))
    Epad = max(P, _ceil_to(e_off[-1], P))
    NKT, NET = Npad // P, Epad // P

    edge_chunks = [_split_chunks(e_off[lv], e_hat[lv]) for lv in range(NL)]
    # target node ptiles per level
    kts = [sorted({r // P for (r, c) in _split_chunks(n_off[lv], n_hat[lv])})
           for lv in range(NL)]
    # 32-aligned window of each (level, ptile): rows [lo, hi) within the
    # ptile live at window rows [r0, r0+w) with r0 in {0,32,64,96}
    win = {}
    for lv in range(NL):
        for kN in kts[lv]:
            lo = max(n_off[lv], kN * P) - kN * P
            hi = min(n_off[lv] + n_hat[lv], (kN + 1) * P) - kN * P
            r0 = min(lo // 32 * 32, 64)   # matmul out base partition <= 64
            w = _ceil_to(hi, 32) - r0
            win[(lv, kN)] = (r0, w, lo, hi)
    # last level writing each ptile (for incremental readout)
    lastlv = {}
    for lv in range(NL):
        for kN in kts[lv]:
            lastlv[kN] = lv

    # commit masks, uniform across cores: one [P,1] mask per (level, ptile)
    mask_idx = {}
    mask_rows = []
    for lv in range(NL):
        for kN in kts[lv]:
            m = np.zeros((P, 1), np.uint8)
            _, _, lo, hi = win[(lv, kN)]
            m[lo:hi, 0] = 1
            mask_idx[(lv, kN)] = len(mask_rows)
            mask_rows.append(m)
    masks = np.stack(mask_rows, axis=0) if mask_rows else np.zeros((1, P, 1), np.uint8)
    NM = masks.shape[0]

    # packed per-(level, edge-chunk, target-ptile) scatter blocks:
    # rows = chunk-local edge, cols = window rows of the target ptile
    afc_col = {}
    ac = 0
    for lv in range(1, NL):
        for ec_i in range(len(edge_chunks[lv])):
            for kN in kts[lv]:
                afc_col[(lv, ec_i, kN)] = ac
                ac += win[(lv, kN)][1]
    AC = max(_ceil_to(ac, 2), 2)

    per_core = []
    for core in range(n_cores):
        slot_of = {}
        xs_idx = np.zeros((Npad, 1), np.int32)
        rel_idx = np.zeros((Npad, 1), np.int32)
        for lv in range(NL):
            for j, (t, s) in enumerate(core_nodes[core][lv]):
                slot = n_off[lv] + j
                g = (core * tpc + t) * S + s
                slot_of[g] = slot
                b = core * tpc + t
                xs_idx[slot, 0] = xs[b, s]
                rel_idx[slot, 0] = rels[b, s]
        G = np.zeros((NKT, P, Epad), np.float32)
        Gp = np.zeros((NKT, P, Epad), np.float32)
        Adj = np.zeros((NKT, P, Npad), np.float32)
        AfcL = np.zeros((P, AC), np.float32)
        Pperm = np.zeros((NKT, P, tpc * S), np.float32)
        for lv in range(1, NL):
            for j, (cg, pg) in enumerate(core_edges[core][lv]):
                e = e_off[lv] + j
                cs, ps = slot_of[cg], slot_of[pg]
                G[cs // P, cs % P, e] = 1.0
                Gp[ps // P, ps % P, e] = 1.0
                Adj[cs // P, cs % P, ps] = 1.0
                for ci_, (erow, ecnt) in enumerate(edge_chunks[lv]):
                    if erow <= e < erow + ecnt:
                        kN = ps // P
                        a0 = afc_col[(lv, ci_, kN)]
                        r0 = win[(lv, kN)][0]
                        AfcL[e - erow, a0 + ps % P - r0] = 1.0
                        break
        for g, slot in slot_of.items():
            t = g // S - core * tpc
            s = g % S
            Pperm[slot // P, slot % P, t * S + s] = 1.0
        per_core.append(dict(xs_idx=xs_idx, rel_idx=rel_idx, G=G, Gp=Gp,
                             Adj=Adj, AfcL=AfcL, Pperm=Pperm))

    # SPMD-uniform nonzero-block flags (OR across cores), level-exact columns
    gnz = np.zeros((NL, NKT), bool)
    for lv in range(1, NL):
        esl = slice(e_off[lv], e_off[lv] + e_hat[lv])
        for k in range(NKT):
            gnz[lv, k] = any(per_core[c]["G"][k, :, esl].any()
                             for c in range(n_cores))
    gpnz = np.zeros((NET, NKT), bool)
    for ke in range(NET):
        esl = slice(ke * P, (ke + 1) * P)
        for k in range(NKT):
            gpnz[ke, k] = any(per_core[c]["Gp"][k, :, esl].any()
                              for c in range(n_cores))

    # combined gather blocks: per level (>=1), cols = [G-cols (even-padded) |
    # window cols of each target ptile]
    ga_off, ga_w, ga_ec2, hoff = {}, {}, {}, {}
    ga_total = 0
    for lv in range(1, NL):
        ec2 = e_hat[lv] + (e_hat[lv] & 1)
        w = ec2
        for kN in kts[lv]:
            hoff[(lv, kN)] = ga_total + w
            w += win[(lv, kN)][1]
        assert w <= 512, f"level {lv} gather block too wide ({w})"
        ga_ec2[lv] = ec2
        ga_off[lv] = ga_total
        ga_w[lv] = w
        ga_total += w
    for cd in per_core:
        GA = np.zeros((NKT, P, max(ga_total, 2)), np.float32)
        for lv in range(1, NL):
            o0, ec2 = ga_off[lv], ga_ec2[lv]
            e0 = e_off[lv]
            for k in range(NKT):
                GA[k, :, o0:o0 + min(ec2, Epad - e0)] = \
                    cd["G"][k][:, e0:e0 + min(ec2, Epad - e0)]
                for kN in kts[lv]:
                    r0, w, lo, hi = win[(lv, kN)]
                    blk = cd["Adj"][k][:, kN * P + r0:kN * P + r0 + w].copy()
                    blk[:, :lo - r0] = 0.0
                    blk[:, hi - r0:] = 0.0
                    o1 = o0 + (hoff[(lv, kN)] - ga_off[lv])
                    GA[k, :, o1:o1 + w] = blk
        cd["GA"] = GA
    GAtot = max(ga_total, 2)
    kgb = np.zeros((NL, NKT), bool)
    for lv in range(1, NL):
        for k in range(NKT):
            kgb[lv, k] = any(
                per_core[c]["GA"][k][:, ga_off[lv]:ga_off[lv] + ga_w[lv]].any()
                for c in range(n_cores))

    # per-ptile slot presence rows for incremental pooling (scaled by the
    # presence offset so absent tree-order columns never win the max)
    for cd in per_core:
        cd["prow"] = (cd["Pperm"].any(axis=1).astype(np.float32)
                      * POOL_OFS)                     # [NKT, TS]

    sizes = dict(NL=NL, Npad=Npad, Epad=Epad, NKT=NKT, NET=NET, tpc=tpc, S=S,
                 AC=AC, NM=NM, n_hat=n_hat, e_hat=e_hat, n_off=n_off,
                 e_off=e_off, edge_chunks=edge_chunks, kts=kts, win=win,
                 lastlv=lastlv, mask_idx=mask_idx, masks=masks,
                 afc_col=afc_col, gnz=gnz, gpnz=gpnz, kgb=kgb,
                 ga_off=ga_off, ga_w=ga_w, ga_ec2=ga_ec2, hoff=hoff,
                 GAtot=GAtot)

    # ---- packed constant column layout (bf16 block + int block) ----
    TS = tpc * S
    cols = {}
    cptr = 0
    def _alloc(name, w):
        nonlocal cptr
        cols[name] = (cptr, w)
        cptr += _ceil_to(w, 2)
    _alloc("bias", G3 + H + 16)       # row0: [bi768 | bf256 | bout]
    _alloc("ones", P)
    _alloc("ident", P)
    _alloc("xT0", Npad)               # host-gathered x^T (emb+rel lookup)
    _alloc("xT1", Npad)
    for k in range(NKT):
        _alloc(f"prow{k}", TS)
    for d in range(2):
        _alloc(f"wioux{d}", G3)
        _alloc(f"wfx{d}", H)
    for k2 in range(2):
        _alloc(f"wiouh{k2}", G3)
        _alloc(f"wfh{k2}", H)
        _alloc(f"wout{k2}", 16)
    for k in range(NKT):
        _alloc(f"GA{k}", GAtot)
    for k in range(NKT):
        _alloc(f"Gp{k}", Epad)
    _alloc("Afc", AC)
    for k in range(NKT):
        _alloc(f"Pp{k}", TS)
    sizes["cols"] = cols
    sizes["C"] = cptr
    icols = {}
    iptr = 0
    def _ialloc(name, w):
        nonlocal iptr
        icols[name] = (iptr, w)
        iptr += w
    _ialloc("xsidx", NKT)
    _ialloc("relidx", NKT)
    _ialloc("masks", NM)
    sizes["icols"] = icols
    sizes["CI"] = iptr
    return sizes, per_core


def pack_weights(inp):
    f32 = np.float32
    a = lambda k: np.asarray(inp[k], f32)
    WiouX = np.ascontiguousarray(
        np.concatenate([a("W_ux"), a("W_ix"), a("W_ox")], axis=1))   # [DIN,768]
    WiouH = np.ascontiguousarray(
        np.concatenate([a("W_uh"), a("W_ih"), a("W_oh")], axis=1))   # [H,768]
    bi512 = np.zeros((1, 512), f32)
    bi512[0, H:2 * H] = a("b_ix") + a("b_ih")
    bf = np.ascontiguousarray((a("b_fx") + a("b_fh")).reshape(1, H))
    return WiouX, WiouH, bi512, bf


# ----------------------------------------------------------------------------
# Numpy emulation of the device program (validation / fallback)
# ----------------------------------------------------------------------------

def emulate_core(sizes, cd, emb_W, rel_W, WiouX, WiouH, Wfx, Wfh,
                 bi512, bf, Wout, bout):
    import ml_dtypes
    bf16 = ml_dtypes.bfloat16
    f32 = np.float32
    b = lambda v: v.astype(bf16)
    NL, NKT = sizes["NL"], sizes["NKT"]
    Npad, TS = sizes["Npad"], sizes["tpc"] * sizes["S"]
    x = np.concatenate([emb_W[cd["xs_idx"][:, 0]], rel_W[cd["rel_idx"][:, 0]]],
                       axis=1).astype(f32)
    xb = b(x)
    iou_x = b(xb.astype(f32) @ b(WiouX).astype(f32)
              + np.concatenate([bi512[0], np.zeros(G3 - 512, f32)]))
    fx = b(xb.astype(f32) @ b(Wfx).astype(f32) + bf[0])
    GpF = np.concatenate([b(cd["Gp"][k]) for k in range(NKT)], axis=0)
    fxe = b(GpF.astype(f32).T @ fx.astype(f32))
    GAF = np.concatenate([b(cd["GA"][k]) for k in range(NKT)], axis=0)
    AfcF = b(cd["AfcL"])
    h = np.zeros((Npad, H), bf16)
    c = np.zeros((Npad, H), bf16)
    WiouHb, Wfhb = b(WiouH), b(Wfh)

    def sigmoid(v):
        return (1.0 / (1.0 + np.exp(-v.astype(f32)))).astype(f32)

    pooled = {}
    for lv in range(NL):
        fc_full = {}
        if lv > 0:
            o0 = sizes["ga_off"][lv]
            gw = sizes["ga_w"][lv]
            ec2 = sizes["ga_ec2"][lv]
            hgst = b(h.astype(f32).T @ GAF[:, o0:o0 + gw].astype(f32))
            for ec_i, (erow, ecnt) in enumerate(sizes["edge_chunks"][lv]):
                eloc = erow - sizes["e_off"][lv]
                cg = (GAF[:, o0 + eloc:o0 + eloc + ecnt].astype(f32).T
                      @ c.astype(f32))
                hch = hgst[:, eloc:eloc + ecnt]
                fpre = (hch.astype(f32).T @ Wfhb.astype(f32)
                        + fxe[erow:erow + ecnt].astype(f32))
                fce = b(b(sigmoid(fpre)).astype(f32) * b(cg).astype(f32))
                for kN in sizes["kts"][lv]:
                    a0 = sizes["afc_col"][(lv, ec_i, kN)]
                    r0, w, _, _ = sizes["win"][(lv, kN)]
                    Af = AfcF[:ecnt, a0:a0 + w]
                    blk = Af.astype(f32).T @ fce.astype(f32)
                    fc_full.setdefault(kN, np.zeros((P, H), f32))[r0:r0 + w] \
                        += blk
        for kN in sizes["kts"][lv]:
            r0, w, _, _ = sizes["win"][(lv, kN)]
            rs = slice(r0, r0 + w)
            iou = iou_x[kN * P:(kN + 1) * P][rs].astype(f32).copy()
            if lv > 0:
                ho = sizes["hoff"][(lv, kN)] - sizes["ga_off"][lv]
                hs = hgst[:, ho:ho + w]
                iou += hs.astype(f32).T @ WiouHb.astype(f32)
            u = np.tanh(iou[:, 0:H]).astype(f32)
            i = sigmoid(iou[:, H:2 * H])
            og = sigmoid(iou[:, 2 * H:])
            cn = (i * u).astype(f32)
            if lv > 0 and kN in fc_full:
                cn = cn + fc_full[kN][rs]
            hn = (og * np.tanh(cn)).astype(f32)
            m = sizes["masks"][sizes["mask_idx"][(lv, kN)]][rs, 0] > 0
            c[kN * P:(kN + 1) * P][rs][m] = b(cn)[m]
            h[kN * P:(kN + 1) * P][rs][m] = b(hn)[m]
        for kN in range(NKT):
            if sizes["lastlv"].get(kN, -1) == lv:
                pr = (h[kN * P:(kN + 1) * P].astype(f32).T
                      @ b(cd["Pperm"][kN]).astype(f32)
                      + b(cd["prow"][kN]).astype(f32)[None, :])
                pooled[kN] = pr

    S = sizes["S"]
    prall = np.stack([pooled[k] for k in sorted(pooled)], axis=0).max(axis=0)
    pool = np.stack([prall[:, t * S:(t + 1) * S].max(axis=1)
                     for t in range(sizes["tpc"])], axis=1) - POOL_OFS
    poolb = b(pool)
    return (b(Wout).astype(f32).T @ poolb.astype(f32)) + bout[:, None]


def kernel_numpy(**inputs):
    sizes, per_core = build_plan(inputs["xs"], inputs["rels"],
                                 inputs["child_idx"], inputs["parent_idx"],
                                 inputs["node_height"], int(inputs["n_levels"]))
    WiouX, WiouH, bi512, bf = pack_weights(inputs)
    emb_W = np.asarray(inputs["emb_W"], np.float32)
    rel_W = np.asarray(inputs["rel_W"], np.float32)
    outs = []
    for cd in per_core:
        lT = emulate_core(sizes, cd, emb_W, rel_W, WiouX, WiouH,
                          np.asarray(inputs["W_fx"], np.float32),
                          np.asarray(inputs["W_fh"], np.float32),
                          bi512, bf,
                          np.asarray(inputs["W_out"], np.float32),
                          np.asarray(inputs["b_out"], np.float32))
        outs.append(lT.T)
    return np.concatenate(outs, axis=0).astype(np.float32)


# ----------------------------------------------------------------------------
# Device program
# ----------------------------------------------------------------------------

def build_bass(sizes, V, DE, RV, DR, L):
    from concourse import bacc, bass, mybir, tile

    f32 = mybir.dt.float32
    bf = mybir.dt.bfloat16
    i32 = mybir.dt.int32
    SIG = mybir.ActivationFunctionType.Sigmoid
    TANH = mybir.ActivationFunctionType.Tanh
    AXX = mybir.AxisListType.X

    NL, Npad, Epad = sizes["NL"], sizes["Npad"], sizes["Epad"]
    NKT, NET, tpc, S = sizes["NKT"], sizes["NET"], sizes["tpc"], sizes["S"]
    NM, C, CI = sizes["NM"], sizes["C"], sizes["CI"]
    cols, icols = sizes["cols"], sizes["icols"]
    DIN = DE + DR
    DT = DIN // P
    TS = tpc * S

    nc = bacc.Bacc("TRN2", target_bir_lowering=False, debug=False)

    d_bigc = nc.dram_tensor("bigc", [P, C], bf, kind="ExternalInput")
    d_bigi = nc.dram_tensor("bigi", [P, max(CI, 1)], i32, kind="ExternalInput")
    d_out = nc.dram_tensor("out", [L, tpc], f32, kind="ExternalOutput")

    with tile.TileContext(nc) as tc:
        with (
            tc.tile_pool(name="const", bufs=1) as cp,
            tc.tile_pool(name="psg", bufs=1, space="PSUM") as ps_g,
            tc.tile_pool(name="pscg", bufs=1, space="PSUM") as ps_cg,
            tc.tile_pool(name="psfp", bufs=1, space="PSUM") as ps_fp,
            tc.tile_pool(name="psfc", bufs=1, space="PSUM") as ps_fc,
            tc.tile_pool(name="psgu", bufs=1, space="PSUM") as ps_gu,
            tc.tile_pool(name="psgi", bufs=1, space="PSUM") as ps_gi,
            tc.tile_pool(name="psgo", bufs=1, space="PSUM") as ps_go,
        ):
            t = lambda shape, dt_, tag: cp.tile(shape, dt_, tag=tag, name=tag)
            bigc = t([P, C], bf, "bigc")
            bigi = t([P, max(CI, 1)], i32, "bigi")

            def cc(name):
                off, w = cols[name]
                return bigc[:, off:off + w]

            def ci(name, j):
                off, _ = icols[name]
                return bigi[:, off + j:off + j + 1]

            wioux = [cc(f"wioux{d}") for d in range(DT)]
            wiouh = [cc(f"wiouh{k}") for k in range(HT)]
            wfx = [cc(f"wfx{d}") for d in range(DT)]
            wfh = [cc(f"wfh{k}") for k in range(HT)]
            wout = [cc(f"wout{k}")[:, :L] for k in range(HT)]
            boff = cols["bias"][0]
            bi_row = bigc[0:1, boff:boff + 512]
            bf_row = bigc[0:1, boff + G3:boff + G3 + H]
            bout_row = bigc[0:1, boff + G3 + H:boff + G3 + H + L]
            ones_row = bigc[0:1, cols["ones"][0]:cols["ones"][0] + P]
            identr = cc("ident")
            GAsb = [cc(f"GA{k}") for k in range(NKT)]
            Gpsb = [cc(f"Gp{k}") for k in range(NKT)]
            Afcsb = cc("Afc")
            Ppsb = [cc(f"Pp{k}") for k in range(NKT)]
            prow = [bigc[0:1, cols[f"prow{k}"][0]:cols[f"prow{k}"][0] + TS]
                    for k in range(NKT)]

            xT = [cc(f"xT{d}") for d in range(DT)]
            ioux = [t([P, G3], bf, f"ioux{k}") for k in range(NKT)]
            fxsb = [t([P, H], bf, f"fx{k}") for k in range(NKT)]
            fxesb = [t([P, H], bf, f"fxe{e}") for e in range(NET)]
            hsb = [[t([P, P], bf, f"h{k}_{kh}") for kh in range(HT)]
                   for k in range(NKT)]
            csb = [t([P, H], bf, f"c{k}") for k in range(NKT)]
            hgst = [t([P, 256], bf, f"hgst{kh}") for kh in range(HT)]
            fgate = t([P, H], bf, "fgate")
            fce = t([P, H], bf, "fce")
            isb = t([P, H], f32, "isb")
            osb = t([P, H], f32, "osb")
            usb = t([P, H], f32, "usb")
            cnew = t([P, H], f32, "cnew")
            thsb = t([P, H], f32, "thsb")
            hnew = t([P, H], f32, "hnew")
            pooled = [t([P, tpc], f32, f"pool{k}") for k in range(HT)]
            ptmp = [t([P, tpc], f32, f"ptmp{k}") for k in range(HT)]
            poolb = [t([P, tpc], bf, f"poolb{k}") for k in range(HT)]
            outsb = t([L, tpc], f32, "outsb")

            # ---- preamble loads, spread across engine DMA queues in need
            # order (each dma_start burns ~0.65us issuing 128 descriptors)
            nc.sync.dma_start(bigi[:], d_bigi[:])
            misc_end = cols[f"prow{NKT-1}"][0] + _ceil_to(cols[f"prow{NKT-1}"][1], 2)
            nc.scalar.dma_start(bigc[:, 0:misc_end], d_bigc[:, 0:misc_end])
            wx_end = cols["wfx1"][0] + _ceil_to(cols["wfx1"][1], 2)
            nc.sync.dma_start(bigc[:, misc_end:wx_end],
                              d_bigc[:, misc_end:wx_end])
            wh_end = cols["wout1"][0] + _ceil_to(cols["wout1"][1], 2)
            nc.scalar.dma_start(bigc[:, wx_end:wh_end],
                                d_bigc[:, wx_end:wh_end])
            ga_end = cols[f"GA{NKT-1}"][0] + _ceil_to(cols[f"GA{NKT-1}"][1], 2)
            nc.sync.dma_start(bigc[:, wh_end:ga_end],
                              d_bigc[:, wh_end:ga_end])
            gp_end = cols[f"Gp{NKT-1}"][0] + _ceil_to(cols[f"Gp{NKT-1}"][1], 2)
            nc.scalar.dma_start(bigc[:, ga_end:gp_end],
                                d_bigc[:, ga_end:gp_end])
            nc.scalar.dma_start(bigc[:, gp_end:C], d_bigc[:, gp_end:C])

            # ---- state init (vector+gpsimd; tiny)
            for k in range(NKT):
                for kh in range(HT):
                    nc.vector.memset(hsb[k][kh][:], 0.0)
                nc.gpsimd.memset(csb[k][:], 0.0)
            nc.vector.memset(fce[:], 0.0)

            # ---- input projections (biases folded in via ones-row matmul)
            for k in range(NKT):
                pa = ps_gu.tile([P, 512], f32, tag="gu", name="gu")
                pb = ps_gi.tile([P, H], f32, tag="gi", name="gi")
                for d in range(DT):
                    nc.tensor.matmul(
                        pa[:], lhsT=xT[d][:, k * P:(k + 1) * P],
                        rhs=wioux[d][:, 0:512], start=(d == 0), stop=False)
                nc.tensor.matmul(pa[:], lhsT=ones_row, rhs=bi_row,
                                 start=False, stop=True)
                for d in range(DT):
                    nc.tensor.matmul(
                        pb[:], lhsT=xT[d][:, k * P:(k + 1) * P],
                        rhs=wioux[d][:, 512:G3], start=(d == 0),
                        stop=(d == DT - 1))
                nc.vector.tensor_copy(out=ioux[k][:, 0:512], in_=pa[:])
                nc.vector.tensor_copy(out=ioux[k][:, 512:G3], in_=pb[:])
                pf = ps_fp.tile([P, H], f32, tag="fp", name="fp")
                for d in range(DT):
                    nc.tensor.matmul(pf[:], lhsT=xT[d][:, k * P:(k + 1) * P],
                                     rhs=wfx[d][:], start=(d == 0), stop=False)
                nc.tensor.matmul(pf[:], lhsT=ones_row, rhs=bf_row,
                                 start=False, stop=True)
                nc.vector.tensor_copy(out=fxsb[k][:], in_=pf[:])

            # ---- fxe: fx gathered per edge slot
            for ke in range(NET):
                ks = [k for k in range(NKT) if sizes["gpnz"][ke, k]]
                if not ks:
                    nc.vector.memset(fxesb[ke][:], 0.0)
                    continue
                pf = ps_fp.tile([P, H], f32, tag="fp", name="fp")
                for i, k in enumerate(ks):
                    nc.tensor.matmul(pf[:],
                                     lhsT=Gpsb[k][:, ke * P:(ke + 1) * P],
                                     rhs=fxsb[k][:],
                                     start=(i == 0), stop=(i == len(ks) - 1))
                nc.vector.tensor_copy(out=fxesb[ke][:], in_=pf[:])

            # ---- levels
            ro_order = sorted(sizes["lastlv"],
                              key=lambda k: (sizes["lastlv"][k], k))

            def emit_readout(kN):
                """Pool ptile kN into the running max (h is final here)."""
                first = (kN == ro_order[0])
                for kh in range(HT):
                    pr = ps_cg.tile([P, 384], f32, tag="cg", name="cg")
                    nc.tensor.matmul(pr[:, 0:TS], lhsT=hsb[kN][kh][:],
                                     rhs=Ppsb[kN][:], start=True, stop=False)
                    nc.tensor.matmul(pr[:, 0:TS], lhsT=ones_row,
                                     rhs=prow[kN], start=False, stop=True)
                    for t_ in range(tpc):
                        dst = pooled[kh] if first else ptmp[kh]
                        nc.vector.reduce_max(dst[:, t_:t_ + 1],
                                             pr[:, t_ * S:(t_ + 1) * S],
                                             axis=AXX)
                    if not first:
                        nc.vector.tensor_max(pooled[kh][:], pooled[kh][:],
                                             ptmp[kh][:])

            for lv in range(NL):
                kts = sizes["kts"][lv]
                fc_ps = {}
                # pre-seed the u/i/o psum tiles with ioux via identity
                # matmuls: PE work that lands in the previous level's tail
                piou = {}
                for kN in kts:
                    if lv == 0:
                        continue
                    r0, w, _, _ = sizes["win"][(lv, kN)]
                    rs = slice(r0, r0 + w)
                    pu = ps_gu.tile([P, H], f32, tag="gu", name="gu")
                    pi_ = ps_gi.tile([P, H], f32, tag="gi", name="gi")
                    po = ps_go.tile([P, H], f32, tag="go", name="go")
                    piou[kN] = (pu, pi_, po)
                    for pt_, c0 in ((pu, 0), (pi_, H), (po, 2 * H)):
                        nc.tensor.matmul(pt_[rs, :],
                                         lhsT=identr[:, r0:r0 + w],
                                         rhs=ioux[kN][:, c0:c0 + H],
                                         start=True, stop=False)
                if lv > 0:
                    fresh = set(sizes["kts"][lv - 1])
                    okey = lambda k: (k in fresh, k)
                    kgb = sorted((k for k in range(NKT) if sizes["kgb"][lv, k]),
                                 key=okey)
                    kg = sorted((k for k in range(NKT) if sizes["gnz"][lv, k]),
                                key=okey)
                    echunks = sizes["edge_chunks"][lv]
                    ga0l = sizes["ga_off"][lv]
                    gawl = sizes["ga_w"][lv]
                    assert gawl <= 256
                    nstale_g = sum(1 for k in kgb if k not in fresh)
                    nstale_c = sum(1 for k in kg if k not in fresh)
                    pc = {}
                    # stale-first emission: all matmuls reading only old
                    # ptiles go ahead of anything touching the fresh commit
                    pg = [ps_g.tile([P, 256], f32, tag=f"gst{kh}",
                                    name="gst") for kh in range(HT)]

                    def g_mm(i):
                        k = kgb[i]
                        for kh in range(HT):
                            nc.tensor.matmul(
                                pg[kh][:, :gawl],
                                lhsT=hsb[k][kh][:],
                                rhs=GAsb[k][:, ga0l:ga0l + gawl],
                                start=(i == 0), stop=(i == len(kgb) - 1))

                    def c_mm(ec_i, i):
                        erow, ecnt = echunks[ec_i]
                        eloc = erow - sizes["e_off"][lv]
                        k = kg[i]
                        nc.tensor.matmul(
                            pc[ec_i][:ecnt, 0:H],
                            lhsT=GAsb[k][:, ga0l + eloc:ga0l + eloc + ecnt],
                            rhs=csb[k][:],
                            start=(i == 0), stop=(i == len(kg) - 1))

                    for i in range(nstale_g):
                        g_mm(i)
                    for ec_i in range(len(echunks)):
                        pc[ec_i] = ps_cg.tile([P, H], f32, tag="cg",
                                              name="cg")
                        for i in range(nstale_c):
                            c_mm(ec_i, i)
                    for i in range(nstale_g, len(kgb)):
                        g_mm(i)
                    nc.vector.tensor_copy(out=hgst[0][:, 0:gawl],
                                          in_=pg[0][:, :gawl])
                    nc.scalar.copy(out=hgst[1][:, 0:gawl],
                                   in_=pg[1][:, :gawl])
                    for ec_i in range(len(echunks)):
                        for i in range(nstale_c, len(kg)):
                            c_mm(ec_i, i)
                    # f-gate per edge chunk, scatter into per-target psum
                    for ec_i, (erow, ecnt) in enumerate(echunks):
                        ke, r0e = erow // P, erow % P
                        eloc = erow - sizes["e_off"][lv]
                        pfp = ps_fp.tile([P, H], f32, tag="fp", name="fp")
                        for kh in range(HT):
                            nc.tensor.matmul(pfp[:ecnt, :],
                                             lhsT=hgst[kh][:, eloc:eloc + ecnt],
                                             rhs=wfh[kh][:],
                                             start=(kh == 0), stop=False)
                        nc.tensor.matmul(pfp[:ecnt, :],
                                         lhsT=identr[:, r0e:r0e + ecnt],
                                         rhs=fxesb[ke][:],
                                         start=False, stop=True)
                        nc.scalar.activation(fgate[:ecnt, :], pfp[:ecnt, :], SIG)
                        nc.vector.tensor_mul(fce[:ecnt, :],
                                             fgate[:ecnt, :], pc[ec_i][:ecnt, :])
                        first = (ec_i == 0)
                        last = (ec_i == len(echunks) - 1)
                        for kN in kts:
                            r0, w, _, _ = sizes["win"][(lv, kN)]
                            if first:
                                fc_ps[kN] = ps_fc.tile([P, H], f32, tag="fc",
                                                       name="fc")
                            a0 = sizes["afc_col"][(lv, ec_i, kN)]
                            nc.tensor.matmul(
                                fc_ps[kN][r0:r0 + w, :],
                                lhsT=Afcsb[:, a0:a0 + w],
                                rhs=fce[:],
                                start=first, stop=last)

                # i/o/u per target ptile (psum pre-seeded with ioux by DVE;
                # recurrent matmuls accumulate on top)
                for kN in kts:
                    r0, w, _, _ = sizes["win"][(lv, kN)]
                    rs = slice(r0, r0 + w)
                    if lv > 0:
                        pu, pi_, po = piou[kN]
                        ho = sizes["hoff"][(lv, kN)] - sizes["ga_off"][lv]
                        for pt_, c0 in ((pu, 0), (pi_, H), (po, 2 * H)):
                            for kh in range(HT):
                                nc.tensor.matmul(
                                    pt_[rs, :],
                                    lhsT=hgst[kh][:, ho:ho + w],
                                    rhs=wiouh[kh][:, c0:c0 + H],
                                    start=False, stop=(kh == HT - 1))
                        nc.scalar.activation(usb[rs, 0:P], pu[rs, 0:P], TANH)
                        nc.scalar.activation(isb[rs, 0:P], pi_[rs, 0:P], SIG)
                        nc.scalar.activation(usb[rs, P:H], pu[rs, P:H], TANH)
                        nc.scalar.activation(isb[rs, P:H], pi_[rs, P:H], SIG)
                        nc.scalar.activation(osb[rs, :], po[rs, :], SIG)
                    else:
                        nc.scalar.activation(usb[rs, :], ioux[kN][rs, 0:H],
                                             TANH)
                        nc.scalar.activation(isb[rs, :], ioux[kN][rs, H:512],
                                             SIG)
                        nc.scalar.activation(osb[rs, :], ioux[kN][rs, 512:G3],
                                             SIG)
                    msk = ci("masks", sizes["mask_idx"][(lv, kN)])
                    for hh in range(HT):
                        hs = slice(hh * P, (hh + 1) * P)
                        nc.vector.tensor_mul(cnew[rs, hs], isb[rs, hs],
                                             usb[rs, hs])
                        if lv > 0:
                            nc.vector.tensor_add(cnew[rs, hs], cnew[rs, hs],
                                                 fc_ps[kN][rs, hs])
                        nc.scalar.activation(thsb[rs, hs], cnew[rs, hs], TANH)
                        nc.vector.tensor_mul(hnew[rs, hs], osb[rs, hs],
                                             thsb[rs, hs])
                        nc.vector.copy_predicated(
                            out=hsb[kN][hh][rs, :],
                            mask=msk[rs].to_broadcast([w, P]),
                            data=hnew[rs, hs])
                    nc.vector.copy_predicated(
                        out=csb[kN][rs, :],
                        mask=msk[rs].to_broadcast([w, H]),
                        data=cnew[rs, :])
                # incremental readout for ptiles finalized at lv-1 (delayed
                # one level so the PE queue never waits on the Pp DMA)
                for kN in range(NKT):
                    if sizes["lastlv"].get(kN, -1) == lv - 1:
                        emit_readout(kN)
            for kN in range(NKT):
                if sizes["lastlv"].get(kN, -1) == NL - 1:
                    emit_readout(kN)

            # ---- readout
            plg = ps_fp.tile([P, H], f32, tag="fp", name="fp")
            for kh in range(HT):
                nc.vector.tensor_scalar_add(poolb[kh][:], pooled[kh][:],
                                            -POOL_OFS)
            for kh in range(HT):
                nc.tensor.matmul(plg[:L, 0:tpc], lhsT=wout[kh],
                                 rhs=poolb[kh][:],
                                 start=(kh == 0), stop=False)
            nc.tensor.matmul(plg[:L, 0:tpc], lhsT=bout_row,
                             rhs=ones_row[:, :tpc], start=False, stop=True)
            nc.vector.tensor_copy(out=outsb[:], in_=plg[:L, 0:tpc])
            nc.sync.dma_start(d_out[:, :], outsb[:])

    nc.compile()
    return nc


def _make_in_maps(sizes, per_core, inputs):
    import ml_dtypes
    f32 = np.float32
    WiouX, WiouH, bi512, bf = pack_weights(inputs)
    cols, C = sizes["cols"], sizes["C"]
    icols, CI = sizes["icols"], sizes["CI"]
    NKT, NM = sizes["NKT"], sizes["NM"]
    L = np.asarray(inputs["W_out"]).shape[1]

    base = np.zeros((P, C), f32)

    def put(name, arr, row0=0):
        off, w = cols[name]
        arr = np.asarray(arr, f32)
        base[row0:row0 + arr.shape[0], off:off + arr.shape[1]] = arr

    for d in range(2):
        put(f"wioux{d}", WiouX[d * P:(d + 1) * P])
        put(f"wfx{d}", np.asarray(inputs["W_fx"], f32)[d * P:(d + 1) * P])
    for k2 in range(2):
        put(f"wiouh{k2}", WiouH[k2 * P:(k2 + 1) * P])
        put(f"wfh{k2}", np.asarray(inputs["W_fh"], f32)[k2 * P:(k2 + 1) * P])
        put(f"wout{k2}", np.asarray(inputs["W_out"], f32)[k2 * P:(k2 + 1) * P])
    brow = np.zeros((1, cols["bias"][1]), f32)
    brow[0, :512] = bi512[0]
    brow[0, G3:G3 + H] = bf[0]
    brow[0, G3 + H:G3 + H + L] = np.asarray(inputs["b_out"], f32)
    put("bias", brow)
    put("ones", np.ones((1, P), f32))
    put("ident", np.eye(P, dtype=f32))
    emb_W = np.asarray(inputs["emb_W"], f32)
    rel_W = np.asarray(inputs["rel_W"], f32)

    ibase = np.zeros((P, max(CI, 1)), np.int32)

    in_maps = []
    for cd in per_core:
        bc = base.copy()
        for k in range(NKT):
            off, w = cols[f"GA{k}"]
            bc[:, off:off + cd["GA"].shape[2]] = cd["GA"][k]
            off, w = cols[f"Gp{k}"]
            bc[:, off:off + w] = cd["Gp"][k]
            off, w = cols[f"Pp{k}"]
            bc[:, off:off + w] = cd["Pperm"][k]
            off, w = cols[f"prow{k}"]
            bc[0, off:off + w] = cd["prow"][k]
        off, w = cols["Afc"]
        bc[:, off:off + cd["AfcL"].shape[1]] = cd["AfcL"]
        x = np.concatenate([emb_W[cd["xs_idx"][:, 0]],
                            rel_W[cd["rel_idx"][:, 0]]], axis=1)  # [Npad, 256]
        bc[:, cols["xT0"][0]:cols["xT0"][0] + x.shape[0]] = x[:, 0:P].T
        bc[:, cols["xT1"][0]:cols["xT1"][0] + x.shape[0]] = x[:, P:2 * P].T
        bi_ = ibase.copy()
        xo = icols["xsidx"][0]
        ro = icols["relidx"][0]
        mo = icols["masks"][0]
        for k in range(NKT):
            bi_[:, xo + k] = cd["xs_idx"][k * P:(k + 1) * P, 0]
            bi_[:, ro + k] = cd["rel_idx"][k * P:(k + 1) * P, 0]
        for m in range(NM):
            bi_[:, mo + m] = sizes["masks"][m][:, 0].astype(np.int32)
        in_maps.append(dict(
            bigc=np.ascontiguousarray(bc.astype(ml_dtypes.bfloat16)),
            bigi=np.ascontiguousarray(bi_),
        ))
    return in_maps


def kernel(**inputs):
    sizes, per_core = build_plan(inputs["xs"], inputs["rels"],
                                 inputs["child_idx"], inputs["parent_idx"],
                                 inputs["node_height"], int(inputs["n_levels"]))
    V, DE = np.asarray(inputs["emb_W"]).shape
    RV, DR = np.asarray(inputs["rel_W"]).shape
    L = np.asarray(inputs["W_out"]).shape[1]
    nc = build_bass(sizes, V, DE, RV, DR, L)
    in_maps = _make_in_maps(sizes, per_core, inputs)

    if os.environ.get("TREELSTM_SIM") == "1":
        from concourse.bass_interp import CoreSim
        ncores = int(os.environ.get("TREELSTM_SIM_CORES", N_CORES))
        outs = []
        for cid in range(ncores):
            sim = CoreSim(nc)
            for name, val in in_maps[cid].items():
                sim.tensor(name)[:] = val
            sim.simulate()
            outs.append(np.array(sim.tensor("out")).T)
        return np.concatenate(outs, axis=0).astype(np.float32)

    from concourse.bass_utils import run_bass_kernel_spmd
    res = run_bass_kernel_spmd(nc, in_maps, core_ids=list(range(N_CORES)),
                               trace=bool(int(os.environ.get("TREELSTM_TRACE", "0"))))
    if getattr(kernel, "_keep_results", False):
        kernel.last_results = res
    out = np.concatenate([r["out"].T for r in res.results], axis=0)
    return out.astype(np.float32)


# revision 35
# speedup vs baseline: 1.0021x; 1.0021x over previous
"""ChildSum TreeLSTM (relational) — Trainium2 Bass kernel, 8 NeuronCores.

Strategy (data-parallel over batch, per sharding hint):
  - 16 trees are split over 8 cores, 2 whole trees per core.
  - Inside each core, nodes are relabeled level-by-level (sorted by tree
    height) so each bottom-up level occupies a contiguous row range of a
    padded node space.  All per-level gathers/scatters become small
    matmuls against host-built 0/1 incidence matrices (exact in fp).
  - bf16 everywhere on the matmul path (tolerance 2e-2; measured err
    ~3e-3).  bf16 streams 1 row/cycle at ANY width, so incidence blocks
    are packed to exact/32-aligned widths instead of 256-col padding.
  - Per-level gate math runs on a 32-aligned partition window of the
    target ptile (engine APs may start at partitions 0/32/64/96), and
    commits into the h/c state are masked with copy_predicated.
  - u/i/o PSUM tiles are pre-seeded with the (hoisted) input projection
    via identity matmuls that land in the previous level's tail; the
    recurrent matmuls accumulate on top.
  - Per-level emission is stale-first so the in-order PE queue drains
    next-level gather work during each level's activation/commit tail.
  - Readout is incremental: a node ptile is pooled as soon as the last
    level writing it commits (+4 presence-offset trick so absent slots
    never win the max).
  - Embedding/rel rows are gathered and transposed on the host (pure
    input indexing, part of the sharding step) and packed into the
    per-core constant block; LSTM weights are replicated to every core.

The SPMD program is identical on all cores; per-core behavior differs
only through input data (index vectors + incidence matrices).  Level
sizes are padded to the max across cores.
"""

import os
import numpy as np

P = 128
H = 256
HT = H // P          # h-state partition tiles
G3 = 3 * H           # packed i|o|u width (768)
N_CORES = 8
POOL_OFS = 4.0       # presence offset for incremental max-pooling


# ----------------------------------------------------------------------------
# Host-side plan builder
# ----------------------------------------------------------------------------

def _ceil_to(x, m):
    return (x + m - 1) // m * m


def _split_chunks(row0, cnt):
    """Split a row range into pieces that don't straddle 128-partition tiles."""
    out = []
    r, remaining = row0, cnt
    while remaining > 0:
        take = min(P - (r % P), remaining)
        out.append((r, take))
        r += take
        remaining -= take
    return out


def build_plan(xs, rels, child_idx, parent_idx, node_height, n_levels,
               n_cores=N_CORES):
    xs = np.asarray(xs)
    rels = np.asarray(rels)
    B, S = xs.shape
    tpc = B // n_cores
    heights = np.asarray(node_height).reshape(B, S)
    ci = np.asarray(child_idx)
    pi = np.asarray(parent_idx)
    NL = min(int(heights.max()) + 1, int(n_levels))

    edges_by_parent = {}
    for c, p in zip(ci.tolist(), pi.tolist()):
        edges_by_parent.setdefault(p, []).append(c)

    core_nodes, core_edges = [], []
    for core in range(n_cores):
        nl = [[] for _ in range(NL)]
        el = [[] for _ in range(NL)]
        for t in range(tpc):
            b = core * tpc + t
            for s in range(S):
                h = int(heights[b, s])
                if h < NL:
                    nl[h].append((t, s))
        for lv in range(1, NL):
            for (t, s) in nl[lv]:
                pg = (core * tpc + t) * S + s
                for cg in edges_by_parent.get(pg, []):
                    el[lv].append((cg, pg))
        core_nodes.append(nl)
        core_edges.append(el)

    n_hat = [max(len(core_nodes[c][lv]) for c in range(n_cores)) for lv in range(NL)]
    e_hat = [max(len(core_edges[c][lv]) for c in range(n_cores)) for lv in range(NL)]
    n_off = [0]
    for v in n_hat:
        n_off.append(n_off[-1] + v)
    e_off = [0]
    for v in e_hat:
        e_off.append(e_off[-1] + v)
    Npad = max(P, _ceil_to(n_off[-1], P))
    Epad = max(P, _ceil_to(e_off[-1], P))
    NKT, NET = Npad // P, Epad // P

    edge_chunks = [_split_chunks(e_off[lv], e_hat[lv]) for lv in range(NL)]
    # target node ptiles per level
    kts = [sorted({r // P for (r, c) in _split_chunks(n_off[lv], n_hat[lv])})
           for lv in range(NL)]
    # 32-aligned window of each (level, ptile): rows [lo, hi) within the
    # ptile live at window rows [r0, r0+w) with r0 in {0,32,64,96}
    win = {}
    for lv in range(NL):
        for kN in kts[lv]:
            lo = max(n_off[lv], kN * P) - kN * P
            hi = min(n_off[lv] + n_hat[lv], (kN + 1) * P) - kN * P
            r0 = min(lo // 32 * 32, 64)   # matmul out base partition <= 64
            w = _ceil_to(hi, 32) - r0
            win[(lv, kN)] = (r0, w, lo, hi)
    # last level writing each ptile (for incremental readout)
    lastlv = {}
    for lv in range(NL):
        for kN in kts[lv]:
            lastlv[kN] = lv

    # commit masks, uniform across cores: one [P,1] mask per (level, ptile)
    mask_idx = {}
    mask_rows = []
    for lv in range(NL):
        for kN in kts[lv]:
            m = np.zeros((P, 1), np.uint8)
            _, _, lo, hi = win[(lv, kN)]
            m[lo:hi, 0] = 1
            mask_idx[(lv, kN)] = len(mask_rows)
            mask_rows.append(m)
    masks = np.stack(mask_rows, axis=0) if mask_rows else np.zeros((1, P, 1), np.uint8)
    NM = masks.shape[0]

    # packed per-(level, edge-chunk, target-ptile) scatter blocks:
    # rows = chunk-local edge, cols = window rows of the target ptile
    afc_col = {}
    ac = 0
    for lv in range(1, NL):
        for ec_i in range(len(edge_chunks[lv])):
            for kN in kts[lv]:
                afc_col[(lv, ec_i, kN)] = ac
                ac += win[(lv, kN)][1]
    AC = max(_ceil_to(ac, 2), 2)

    per_core = []
    for core in range(n_cores):
        slot_of = {}
        xs_idx = np.zeros((Npad, 1), np.int32)
        rel_idx = np.zeros((Npad, 1), np.int32)
        for lv in range(NL):
            for j, (t, s) in enumerate(core_nodes[core][lv]):
                slot = n_off[lv] + j
                g = (core * tpc + t) * S + s
                slot_of[g] = slot
                b = core * tpc + t
                xs_idx[slot, 0] = xs[b, s]
                rel_idx[slot, 0] = rels[b, s]
        G = np.zeros((NKT, P, Epad), np.float32)
        Gp = np.zeros((NKT, P, Epad), np.float32)
        Adj = np.zeros((NKT, P, Npad), np.float32)
        AfcL = np.zeros((P, AC), np.float32)
        Pperm = np.zeros((NKT, P, tpc * S), np.float32)
        for lv in range(1, NL):
            for j, (cg, pg) in enumerate(core_edges[core][lv]):
                e = e_off[lv] + j
                cs, ps = slot_of[cg], slot_of[pg]
                G[cs // P, cs % P, e] = 1.0
                Gp[ps // P, ps % P, e] = 1.0
                Adj[cs // P, cs % P, ps] = 1.0
                for ci_, (erow, ecnt) in enumerate(edge_chunks[lv]):
                    if erow <= e < erow + ecnt:
                        kN = ps // P
                        a0 = afc_col[(lv, ci_, kN)]
                        r0 = win[(lv, kN)][0]
                        AfcL[e - erow, a0 + ps % P - r0] = 1.0
                        break
        for g, slot in slot_of.items():
            t = g // S - core * tpc
            s = g % S
            Pperm[slot // P, slot % P, t * S + s] = 1.0
        per_core.append(dict(xs_idx=xs_idx, rel_idx=rel_idx, G=G, Gp=Gp,
                             Adj=Adj, AfcL=AfcL, Pperm=Pperm))

    # SPMD-uniform nonzero-block flags (OR across cores), level-exact columns
    gnz = np.zeros((NL, NKT), bool)
    for lv in range(1, NL):
        esl = slice(e_off[lv], e_off[lv] + e_hat[lv])
        for k in range(NKT):
            gnz[lv, k] = any(per_core[c]["G"][k, :, esl].any()
                             for c in range(n_cores))
    gpnz = np.zeros((NET, NKT), bool)
    for ke in range(NET):
        esl = slice(ke * P, (ke + 1) * P)
        for k in range(NKT):
            gpnz[ke, k] = any(per_core[c]["Gp"][k, :, esl].any()
                              for c in range(n_cores))

    # combined gather blocks: per level (>=1), cols = [G-cols (even-padded) |
    # window cols of each target ptile]
    ga_off, ga_w, ga_ec2, hoff = {}, {}, {}, {}
    ga_total = 0
    for lv in range(1, NL):
        ec2 = e_hat[lv] + (e_hat[lv] & 1)
        w = ec2
        for kN in kts[lv]:
            hoff[(lv, kN)] = ga_total + w
            w += win[(lv, kN)][1]
        assert w <= 512, f"level {lv} gather block too wide ({w})"
        ga_ec2[lv] = ec2
        ga_off[lv] = ga_total
        ga_w[lv] = w
        ga_total += w
    for cd in per_core:
        GA = np.zeros((NKT, P, max(ga_total, 2)), np.float32)
        for lv in range(1, NL):
            o0, ec2 = ga_off[lv], ga_ec2[lv]
            e0 = e_off[lv]
            for k in range(NKT):
                GA[k, :, o0:o0 + min(ec2, Epad - e0)] = \
                    cd["G"][k][:, e0:e0 + min(ec2, Epad - e0)]
                for kN in kts[lv]:
                    r0, w, lo, hi = win[(lv, kN)]
                    blk = cd["Adj"][k][:, kN * P + r0:kN * P + r0 + w].copy()
                    blk[:, :lo - r0] = 0.0
                    blk[:, hi - r0:] = 0.0
                    o1 = o0 + (hoff[(lv, kN)] - ga_off[lv])
                    GA[k, :, o1:o1 + w] = blk
        cd["GA"] = GA
    GAtot = max(ga_total, 2)
    kgb = np.zeros((NL, NKT), bool)
    for lv in range(1, NL):
        for k in range(NKT):
            kgb[lv, k] = any(
                per_core[c]["GA"][k][:, ga_off[lv]:ga_off[lv] + ga_w[lv]].any()
                for c in range(n_cores))

    # per-ptile slot presence rows for incremental pooling (scaled by the
    # presence offset so absent tree-order columns never win the max)
    for cd in per_core:
        cd["prow"] = (cd["Pperm"].any(axis=1).astype(np.float32)
                      * POOL_OFS)                     # [NKT, TS]

    sizes = dict(NL=NL, Npad=Npad, Epad=Epad, NKT=NKT, NET=NET, tpc=tpc, S=S,
                 AC=AC, NM=NM, n_hat=n_hat, e_hat=e_hat, n_off=n_off,
                 e_off=e_off, edge_chunks=edge_chunks, kts=kts, win=win,
                 lastlv=lastlv, mask_idx=mask_idx, masks=masks,
                 afc_col=afc_col, gnz=gnz, gpnz=gpnz, kgb=kgb,
                 ga_off=ga_off, ga_w=ga_w, ga_ec2=ga_ec2, hoff=hoff,
                 GAtot=GAtot)

    # ---- packed constant column layout (bf16 block + int block) ----
    TS = tpc * S
    cols = {}
    cptr = 0
    def _alloc(name, w):
        nonlocal cptr
        cols[name] = (cptr, w)
        cptr += _ceil_to(w, 2)
    _alloc("bias", G3 + H + 16)       # row0: [bi768 | bf256 | bout]
    _alloc("ones", P)
    _alloc("ident", P)
    _alloc("xT0", Npad)               # host-gathered x^T (emb+rel lookup)
    _alloc("xT1", Npad)
    for k in range(NKT):
        _alloc(f"prow{k}", TS)
    for d in range(2):
        _alloc(f"wioux{d}", G3)
        _alloc(f"wfx{d}", H)
    for k2 in range(2):
        _alloc(f"wiouh{k2}", G3)
        _alloc(f"wfh{k2}", H)
        _alloc(f"wout{k2}", 16)
    for k in range(NKT):
        _alloc(f"GA{k}", GAtot)
    for k in range(NKT):
        _alloc(f"Gp{k}", Epad)
    _alloc("Afc", AC)
    for k in range(NKT):
        _alloc(f"Pp{k}", TS)
    sizes["cols"] = cols
    sizes["C"] = cptr
    icols = {}
    iptr = 0
    def _ialloc(name, w):
        nonlocal iptr
        icols[name] = (iptr, w)
        iptr += w
    _ialloc("xsidx", NKT)
    _ialloc("relidx", NKT)
    _ialloc("masks", NM)
    sizes["icols"] = icols
    sizes["CI"] = iptr
    return sizes, per_core


def pack_weights(inp):
    f32 = np.float32
    a = lambda k: np.asarray(inp[k], f32)
    WiouX = np.ascontiguousarray(
        np.concatenate([a("W_ux"), a("W_ix"), a("W_ox")], axis=1))   # [DIN,768]
    WiouH = np.ascontiguousarray(
        np.concatenate([a("W_uh"), a("W_ih"), a("W_oh")], axis=1))   # [H,768]
    bi512 = np.zeros((1, 512), f32)
    bi512[0, H:2 * H] = a("b_ix") + a("b_ih")
    bf = np.ascontiguousarray((a("b_fx") + a("b_fh")).reshape(1, H))
    return WiouX, WiouH, bi512, bf


# ----------------------------------------------------------------------------
# Numpy emulation of the device program (validation / fallback)
# ----------------------------------------------------------------------------

def emulate_core(sizes, cd, emb_W, rel_W, WiouX, WiouH, Wfx, Wfh,
                 bi512, bf, Wout, bout):
    import ml_dtypes
    bf16 = ml_dtypes.bfloat16
    f32 = np.float32
    b = lambda v: v.astype(bf16)
    NL, NKT = sizes["NL"], sizes["NKT"]
    Npad, TS = sizes["Npad"], sizes["tpc"] * sizes["S"]
    x = np.concatenate([emb_W[cd["xs_idx"][:, 0]], rel_W[cd["rel_idx"][:, 0]]],
                       axis=1).astype(f32)
    xb = b(x)
    iou_x = b(xb.astype(f32) @ b(WiouX).astype(f32)
              + np.concatenate([bi512[0], np.zeros(G3 - 512, f32)]))
    fx = b(xb.astype(f32) @ b(Wfx).astype(f32) + bf[0])
    GpF = np.concatenate([b(cd["Gp"][k]) for k in range(NKT)], axis=0)
    fxe = b(GpF.astype(f32).T @ fx.astype(f32))
    GAF = np.concatenate([b(cd["GA"][k]) for k in range(NKT)], axis=0)
    AfcF = b(cd["AfcL"])
    h = np.zeros((Npad, H), bf16)
    c = np.zeros((Npad, H), bf16)
    WiouHb, Wfhb = b(WiouH), b(Wfh)

    def sigmoid(v):
        return (1.0 / (1.0 + np.exp(-v.astype(f32)))).astype(f32)

    pooled = {}
    for lv in range(NL):
        fc_full = {}
        if lv > 0:
            o0 = sizes["ga_off"][lv]
            gw = sizes["ga_w"][lv]
            ec2 = sizes["ga_ec2"][lv]
            hgst = b(h.astype(f32).T @ GAF[:, o0:o0 + gw].astype(f32))
            for ec_i, (erow, ecnt) in enumerate(sizes["edge_chunks"][lv]):
                eloc = erow - sizes["e_off"][lv]
                cg = (GAF[:, o0 + eloc:o0 + eloc + ecnt].astype(f32).T
                      @ c.astype(f32))
                hch = hgst[:, eloc:eloc + ecnt]
                fpre = (hch.astype(f32).T @ Wfhb.astype(f32)
                        + fxe[erow:erow + ecnt].astype(f32))
                fce = b(b(sigmoid(fpre)).astype(f32) * b(cg).astype(f32))
                for kN in sizes["kts"][lv]:
                    a0 = sizes["afc_col"][(lv, ec_i, kN)]
                    r0, w, _, _ = sizes["win"][(lv, kN)]
                    Af = AfcF[:ecnt, a0:a0 + w]
                    blk = Af.astype(f32).T @ fce.astype(f32)
                    fc_full.setdefault(kN, np.zeros((P, H), f32))[r0:r0 + w] \
                        += blk
        for kN in sizes["kts"][lv]:
            r0, w, _, _ = sizes["win"][(lv, kN)]
            rs = slice(r0, r0 + w)
            iou = iou_x[kN * P:(kN + 1) * P][rs].astype(f32).copy()
            if lv > 0:
                ho = sizes["hoff"][(lv, kN)] - sizes["ga_off"][lv]
                hs = hgst[:, ho:ho + w]
                iou += hs.astype(f32).T @ WiouHb.astype(f32)
            u = np.tanh(iou[:, 0:H]).astype(f32)
            i = sigmoid(iou[:, H:2 * H])
            og = sigmoid(iou[:, 2 * H:])
            cn = (i * u).astype(f32)
            if lv > 0 and kN in fc_full:
                cn = cn + fc_full[kN][rs]
            hn = (og * np.tanh(cn)).astype(f32)
            m = sizes["masks"][sizes["mask_idx"][(lv, kN)]][rs, 0] > 0
            c[kN * P:(kN + 1) * P][rs][m] = b(cn)[m]
            h[kN * P:(kN + 1) * P][rs][m] = b(hn)[m]
        for kN in range(NKT):
            if sizes["lastlv"].get(kN, -1) == lv:
                pr = (h[kN * P:(kN + 1) * P].astype(f32).T
                      @ b(cd["Pperm"][kN]).astype(f32)
                      + b(cd["prow"][kN]).astype(f32)[None, :])
                pooled[kN] = pr

    S = sizes["S"]
    prall = np.stack([pooled[k] for k in sorted(pooled)], axis=0).max(axis=0)
    pool = np.stack([prall[:, t * S:(t + 1) * S].max(axis=1)
                     for t in range(sizes["tpc"])], axis=1) - POOL_OFS
    poolb = b(pool)
    return (b(Wout).astype(f32).T @ poolb.astype(f32)) + bout[:, None]


def kernel_numpy(**inputs):
    sizes, per_core = build_plan(inputs["xs"], inputs["rels"],
                                 inputs["child_idx"], inputs["parent_idx"],
                                 inputs["node_height"], int(inputs["n_levels"]))
    WiouX, WiouH, bi512, bf = pack_weights(inputs)
    emb_W = np.asarray(inputs["emb_W"], np.float32)
    rel_W = np.asarray(inputs["rel_W"], np.float32)
    outs = []
    for cd in per_core:
        lT = emulate_core(sizes, cd, emb_W, rel_W, WiouX, WiouH,
                          np.asarray(inputs["W_fx"], np.float32),
                          np.asarray(inputs["W_fh"], np.float32),
                          bi512, bf,
                          np.asarray(inputs["W_out"], np.float32),
                          np.asarray(inputs["b_out"], np.float32))
        outs.append(lT.T)
    return np.concatenate(outs, axis=0).astype(np.float32)


# ----------------------------------------------------------------------------
# Device program
# ----------------------------------------------------------------------------

def build_bass(sizes, V, DE, RV, DR, L):
    from concourse import bacc, bass, mybir, tile

    f32 = mybir.dt.float32
    bf = mybir.dt.bfloat16
    i32 = mybir.dt.int32
    SIG = mybir.ActivationFunctionType.Sigmoid
    TANH = mybir.ActivationFunctionType.Tanh
    AXX = mybir.AxisListType.X

    NL, Npad, Epad = sizes["NL"], sizes["Npad"], sizes["Epad"]
    NKT, NET, tpc, S = sizes["NKT"], sizes["NET"], sizes["tpc"], sizes["S"]
    NM, C, CI = sizes["NM"], sizes["C"], sizes["CI"]
    cols, icols = sizes["cols"], sizes["icols"]
    DIN = DE + DR
    DT = DIN // P
    TS = tpc * S

    nc = bacc.Bacc("TRN2", target_bir_lowering=False, debug=False)

    d_bigc = nc.dram_tensor("bigc", [P, C], bf, kind="ExternalInput")
    d_bigi = nc.dram_tensor("bigi", [P, max(CI, 1)], i32, kind="ExternalInput")
    d_out = nc.dram_tensor("out", [L, tpc], f32, kind="ExternalOutput")

    with tile.TileContext(nc) as tc:
        with (
            tc.tile_pool(name="const", bufs=1) as cp,
            tc.tile_pool(name="psg", bufs=1, space="PSUM") as ps_g,
            tc.tile_pool(name="pscg", bufs=1, space="PSUM") as ps_cg,
            tc.tile_pool(name="psfp", bufs=1, space="PSUM") as ps_fp,
            tc.tile_pool(name="psfc", bufs=1, space="PSUM") as ps_fc,
            tc.tile_pool(name="psgu", bufs=1, space="PSUM") as ps_gu,
            tc.tile_pool(name="psgi", bufs=1, space="PSUM") as ps_gi,
            tc.tile_pool(name="psgo", bufs=1, space="PSUM") as ps_go,
        ):
            t = lambda shape, dt_, tag: cp.tile(shape, dt_, tag=tag, name=tag)
            bigc = t([P, C], bf, "bigc")
            bigi = t([P, max(CI, 1)], i32, "bigi")

            def cc(name):
                off, w = cols[name]
                return bigc[:, off:off + w]

            def ci(name, j):
                off, _ = icols[name]
                return bigi[:, off + j:off + j + 1]

            wioux = [cc(f"wioux{d}") for d in range(DT)]
            wiouh = [cc(f"wiouh{k}") for k in range(HT)]
            wfx = [cc(f"wfx{d}") for d in range(DT)]
            wfh = [cc(f"wfh{k}") for k in range(HT)]
            wout = [cc(f"wout{k}")[:, :L] for k in range(HT)]
            boff = cols["bias"][0]
            bi_row = bigc[0:1, boff:boff + 512]
            bf_row = bigc[0:1, boff + G3:boff + G3 + H]
            bout_row = bigc[0:1, boff + G3 + H:boff + G3 + H + L]
            ones_row = bigc[0:1, cols["ones"][0]:cols["ones"][0] + P]
            identr = cc("ident")
            GAsb = [cc(f"GA{k}") for k in range(NKT)]
            Gpsb = [cc(f"Gp{k}") for k in range(NKT)]
            Afcsb = cc("Afc")
            Ppsb = [cc(f"Pp{k}") for k in range(NKT)]
            prow = [bigc[0:1, cols[f"prow{k}"][0]:cols[f"prow{k}"][0] + TS]
                    for k in range(NKT)]

            xT = [cc(f"xT{d}") for d in range(DT)]
            ioux = [t([P, G3], bf, f"ioux{k}") for k in range(NKT)]
            fxsb = [t([P, H], bf, f"fx{k}") for k in range(NKT)]
            fxesb = [t([P, H], bf, f"fxe{e}") for e in range(NET)]
            hsb = [[t([P, P], bf, f"h{k}_{kh}") for kh in range(HT)]
                   for k in range(NKT)]
            csb = [t([P, H], bf, f"c{k}") for k in range(NKT)]
            hgst = [t([P, 256], bf, f"hgst{kh}") for kh in range(HT)]
            fgate = t([P, H], bf, "fgate")
            fce = t([P, H], bf, "fce")
            isb = t([P, H], f32, "isb")
            osb = t([P, H], f32, "osb")
            usb = t([P, H], f32, "usb")
            cnew = t([P, H], f32, "cnew")
            thsb = t([P, H], f32, "thsb")
            hnew = t([P, H], f32, "hnew")
            pooled = [t([P, tpc], f32, f"pool{k}") for k in range(HT)]
            ptmp = [t([P, tpc], f32, f"ptmp{k}") for k in range(HT)]
            poolb = [t([P, tpc], bf, f"poolb{k}") for k in range(HT)]
            outsb = t([L, tpc], f32, "outsb")

            # ---- preamble loads, spread across engine DMA queues in need
            # order (each dma_start burns ~0.65us issuing 128 descriptors)
            misc_end = cols[f"prow{NKT-1}"][0] + _ceil_to(cols[f"prow{NKT-1}"][1], 2)
            nc.scalar.dma_start(bigc[:, 0:misc_end], d_bigc[:, 0:misc_end])
            wx_end = cols["wfx1"][0] + _ceil_to(cols["wfx1"][1], 2)
            nc.sync.dma_start(bigc[:, misc_end:wx_end],
                              d_bigc[:, misc_end:wx_end])
            nc.sync.dma_start(bigi[:], d_bigi[:])
            wh_end = cols["wout1"][0] + _ceil_to(cols["wout1"][1], 2)
            nc.scalar.dma_start(bigc[:, wx_end:wh_end],
                                d_bigc[:, wx_end:wh_end])
            ga_end = cols[f"GA{NKT-1}"][0] + _ceil_to(cols[f"GA{NKT-1}"][1], 2)
            nc.sync.dma_start(bigc[:, wh_end:ga_end],
                              d_bigc[:, wh_end:ga_end])
            gp_end = cols[f"Gp{NKT-1}"][0] + _ceil_to(cols[f"Gp{NKT-1}"][1], 2)
            nc.scalar.dma_start(bigc[:, ga_end:gp_end],
                                d_bigc[:, ga_end:gp_end])
            nc.scalar.dma_start(bigc[:, gp_end:C], d_bigc[:, gp_end:C])

            # ---- state init (vector+gpsimd; tiny)
            for k in range(NKT):
                for kh in range(HT):
                    nc.vector.memset(hsb[k][kh][:], 0.0)
                nc.gpsimd.memset(csb[k][:], 0.0)
            nc.vector.memset(fce[:], 0.0)

            # ---- input projections (biases folded in via ones-row matmul)
            for k in range(NKT):
                pa = ps_gu.tile([P, 512], f32, tag="gu", name="gu")
                pb = ps_gi.tile([P, H], f32, tag="gi", name="gi")
                for d in range(DT):
                    nc.tensor.matmul(
                        pa[:], lhsT=xT[d][:, k * P:(k + 1) * P],
                        rhs=wioux[d][:, 0:512], start=(d == 0), stop=False)
                nc.tensor.matmul(pa[:], lhsT=ones_row, rhs=bi_row,
                                 start=False, stop=True)
                for d in range(DT):
                    nc.tensor.matmul(
                        pb[:], lhsT=xT[d][:, k * P:(k + 1) * P],
                        rhs=wioux[d][:, 512:G3], start=(d == 0),
                        stop=(d == DT - 1))
                nc.vector.tensor_copy(out=ioux[k][:, 0:512], in_=pa[:])
                nc.vector.tensor_copy(out=ioux[k][:, 512:G3], in_=pb[:])
                pf = ps_fp.tile([P, H], f32, tag="fp", name="fp")
                for d in range(DT):
                    nc.tensor.matmul(pf[:], lhsT=xT[d][:, k * P:(k + 1) * P],
                                     rhs=wfx[d][:], start=(d == 0), stop=False)
                nc.tensor.matmul(pf[:], lhsT=ones_row, rhs=bf_row,
                                 start=False, stop=True)
                nc.vector.tensor_copy(out=fxsb[k][:], in_=pf[:])

            # ---- fxe: fx gathered per edge slot
            for ke in range(NET):
                ks = [k for k in range(NKT) if sizes["gpnz"][ke, k]]
                if not ks:
                    nc.vector.memset(fxesb[ke][:], 0.0)
                    continue
                pf = ps_fp.tile([P, H], f32, tag="fp", name="fp")
                for i, k in enumerate(ks):
                    nc.tensor.matmul(pf[:],
                                     lhsT=Gpsb[k][:, ke * P:(ke + 1) * P],
                                     rhs=fxsb[k][:],
                                     start=(i == 0), stop=(i == len(ks) - 1))
                nc.vector.tensor_copy(out=fxesb[ke][:], in_=pf[:])

            # ---- levels
            ro_order = sorted(sizes["lastlv"],
                              key=lambda k: (sizes["lastlv"][k], k))

            def emit_readout(kN):
                """Pool ptile kN into the running max (h is final here)."""
                first = (kN == ro_order[0])
                for kh in range(HT):
                    pr = ps_cg.tile([P, 384], f32, tag="cg", name="cg")
                    nc.tensor.matmul(pr[:, 0:TS], lhsT=hsb[kN][kh][:],
                                     rhs=Ppsb[kN][:], start=True, stop=False)
                    nc.tensor.matmul(pr[:, 0:TS], lhsT=ones_row,
                                     rhs=prow[kN], start=False, stop=True)
                    for t_ in range(tpc):
                        dst = pooled[kh] if first else ptmp[kh]
                        nc.vector.reduce_max(dst[:, t_:t_ + 1],
                                             pr[:, t_ * S:(t_ + 1) * S],
                                             axis=AXX)
                    if not first:
                        nc.vector.tensor_max(pooled[kh][:], pooled[kh][:],
                                             ptmp[kh][:])

            for lv in range(NL):
                kts = sizes["kts"][lv]
                fc_ps = {}
                # pre-seed the u/i/o psum tiles with ioux via identity
                # matmuls: PE work that lands in the previous level's tail
                piou = {}
                for kN in kts:
                    if lv == 0:
                        continue
                    r0, w, _, _ = sizes["win"][(lv, kN)]
                    rs = slice(r0, r0 + w)
                    pu = ps_gu.tile([P, H], f32, tag="gu", name="gu")
                    pi_ = ps_gi.tile([P, H], f32, tag="gi", name="gi")
                    po = ps_go.tile([P, H], f32, tag="go", name="go")
                    piou[kN] = (pu, pi_, po)
                    for pt_, c0 in ((pu, 0), (pi_, H), (po, 2 * H)):
                        nc.tensor.matmul(pt_[rs, :],
                                         lhsT=identr[:, r0:r0 + w],
                                         rhs=ioux[kN][:, c0:c0 + H],
                                         start=True, stop=False)
                if lv > 0:
                    fresh = set(sizes["kts"][lv - 1])
                    okey = lambda k: (k in fresh, k)
                    kgb = sorted((k for k in range(NKT) if sizes["kgb"][lv, k]),
                                 key=okey)
                    kg = sorted((k for k in range(NKT) if sizes["gnz"][lv, k]),
                                key=okey)
                    echunks = sizes["edge_chunks"][lv]
                    ga0l = sizes["ga_off"][lv]
                    gawl = sizes["ga_w"][lv]
                    assert gawl <= 256
                    nstale_g = sum(1 for k in kgb if k not in fresh)
                    nstale_c = sum(1 for k in kg if k not in fresh)
                    pc = {}
                    # stale-first emission: all matmuls reading only old
                    # ptiles go ahead of anything touching the fresh commit
                    pg = [ps_g.tile([P, 256], f32, tag=f"gst{kh}",
                                    name="gst") for kh in range(HT)]

                    def g_mm(i):
                        k = kgb[i]
                        for kh in range(HT):
                            nc.tensor.matmul(
                                pg[kh][:, :gawl],
                                lhsT=hsb[k][kh][:],
                                rhs=GAsb[k][:, ga0l:ga0l + gawl],
                                start=(i == 0), stop=(i == len(kgb) - 1))

                    def c_mm(ec_i, i):
                        erow, ecnt = echunks[ec_i]
                        eloc = erow - sizes["e_off"][lv]
                        k = kg[i]
                        nc.tensor.matmul(
                            pc[ec_i][:ecnt, 0:H],
                            lhsT=GAsb[k][:, ga0l + eloc:ga0l + eloc + ecnt],
                            rhs=csb[k][:],
                            start=(i == 0), stop=(i == len(kg) - 1))

                    for i in range(nstale_g):
                        g_mm(i)
                    for ec_i in range(len(echunks)):
                        pc[ec_i] = ps_cg.tile([P, H], f32, tag="cg",
                                              name="cg")
                        for i in range(nstale_c):
                            c_mm(ec_i, i)
                    for i in range(nstale_g, len(kgb)):
                        g_mm(i)
                    nc.vector.tensor_copy(out=hgst[0][:, 0:gawl],
                                          in_=pg[0][:, :gawl])
                    nc.scalar.copy(out=hgst[1][:, 0:gawl],
                                   in_=pg[1][:, :gawl])
                    for ec_i in range(len(echunks)):
                        for i in range(nstale_c, len(kg)):
                            c_mm(ec_i, i)
                    # f-gate per edge chunk, scatter into per-target psum
                    for ec_i, (erow, ecnt) in enumerate(echunks):
                        ke, r0e = erow // P, erow % P
                        eloc = erow - sizes["e_off"][lv]
                        pfp = ps_fp.tile([P, H], f32, tag="fp", name="fp")
                        for kh in range(HT):
                            nc.tensor.matmul(pfp[:ecnt, :],
                                             lhsT=hgst[kh][:, eloc:eloc + ecnt],
                                             rhs=wfh[kh][:],
                                             start=(kh == 0), stop=False)
                        nc.tensor.matmul(pfp[:ecnt, :],
                                         lhsT=identr[:, r0e:r0e + ecnt],
                                         rhs=fxesb[ke][:],
                                         start=False, stop=True)
                        nc.scalar.activation(fgate[:ecnt, :], pfp[:ecnt, :], SIG)
                        nc.vector.tensor_mul(fce[:ecnt, :],
                                             fgate[:ecnt, :], pc[ec_i][:ecnt, :])
                        first = (ec_i == 0)
                        last = (ec_i == len(echunks) - 1)
                        for kN in kts:
                            r0, w, _, _ = sizes["win"][(lv, kN)]
                            if first:
                                fc_ps[kN] = ps_fc.tile([P, H], f32, tag="fc",
                                                       name="fc")
                            a0 = sizes["afc_col"][(lv, ec_i, kN)]
                            nc.tensor.matmul(
                                fc_ps[kN][r0:r0 + w, :],
                                lhsT=Afcsb[:, a0:a0 + w],
                                rhs=fce[:],
                                start=first, stop=last)

                # i/o/u per target ptile (psum pre-seeded with ioux by DVE;
                # recurrent matmuls accumulate on top)
                for kN in kts:
                    r0, w, _, _ = sizes["win"][(lv, kN)]
                    rs = slice(r0, r0 + w)
                    if lv > 0:
                        pu, pi_, po = piou[kN]
                        ho = sizes["hoff"][(lv, kN)] - sizes["ga_off"][lv]
                        for pt_, c0 in ((pu, 0), (pi_, H), (po, 2 * H)):
                            for kh in range(HT):
                                nc.tensor.matmul(
                                    pt_[rs, :],
                                    lhsT=hgst[kh][:, ho:ho + w],
                                    rhs=wiouh[kh][:, c0:c0 + H],
                                    start=False, stop=(kh == HT - 1))
                        nc.scalar.activation(usb[rs, 0:P], pu[rs, 0:P], TANH)
                        nc.scalar.activation(isb[rs, 0:P], pi_[rs, 0:P], SIG)
                        nc.scalar.activation(usb[rs, P:H], pu[rs, P:H], TANH)
                        nc.scalar.activation(isb[rs, P:H], pi_[rs, P:H], SIG)
                        nc.scalar.activation(osb[rs, :], po[rs, :], SIG)
                    else:
                        nc.scalar.activation(usb[rs, :], ioux[kN][rs, 0:H],
                                             TANH)
                        nc.scalar.activation(isb[rs, :], ioux[kN][rs, H:512],
                                             SIG)
                        nc.scalar.activation(osb[rs, :], ioux[kN][rs, 512:G3],
                                             SIG)
                    msk = ci("masks", sizes["mask_idx"][(lv, kN)])
                    for hh in range(HT):
                        hs = slice(hh * P, (hh + 1) * P)
                        nc.vector.tensor_mul(cnew[rs, hs], isb[rs, hs],
                                             usb[rs, hs])
                        if lv > 0:
                            nc.vector.tensor_add(cnew[rs, hs], cnew[rs, hs],
                                                 fc_ps[kN][rs, hs])
                        nc.scalar.activation(thsb[rs, hs], cnew[rs, hs], TANH)
                        nc.vector.tensor_mul(hnew[rs, hs], osb[rs, hs],
                                             thsb[rs, hs])
                        nc.vector.copy_predicated(
                            out=hsb[kN][hh][rs, :],
                            mask=msk[rs].to_broadcast([w, P]),
                            data=hnew[rs, hs])
                    nc.vector.copy_predicated(
                        out=csb[kN][rs, :],
                        mask=msk[rs].to_broadcast([w, H]),
                        data=cnew[rs, :])
                # incremental readout for ptiles finalized at lv-1 (delayed
                # one level so the PE queue never waits on the Pp DMA)
                for kN in range(NKT):
                    if sizes["lastlv"].get(kN, -1) == lv - 1:
                        emit_readout(kN)
            for kN in range(NKT):
                if sizes["lastlv"].get(kN, -1) == NL - 1:
                    emit_readout(kN)

            # ---- readout
            plg = ps_fp.tile([P, H], f32, tag="fp", name="fp")
            for kh in range(HT):
                nc.vector.tensor_scalar_add(poolb[kh][:], pooled[kh][:],
                                            -POOL_OFS)
            for kh in range(HT):
                nc.tensor.matmul(plg[:L, 0:tpc], lhsT=wout[kh],
                                 rhs=poolb[kh][:],
                                 start=(kh == 0), stop=False)
            nc.tensor.matmul(plg[:L, 0:tpc], lhsT=bout_row,
                             rhs=ones_row[:, :tpc], start=False, stop=True)
            nc.vector.tensor_copy(out=outsb[:], in_=plg[:L, 0:tpc])
            nc.sync.dma_start(d_out[:, :], outsb[:])

    nc.compile()
    return nc


def _make_in_maps(sizes, per_core, inputs):
    import ml_dtypes
    f32 = np.float32
    WiouX, WiouH, bi512, bf = pack_weights(inputs)
    cols, C = sizes["cols"], sizes["C"]
    icols, CI = sizes["icols"], sizes["CI"]
    NKT, NM = sizes["NKT"], sizes["NM"]
    L = np.asarray(inputs["W_out"]).shape[1]

    base = np.zeros((P, C), f32)

    def put(name, arr, row0=0):
        off, w = cols[name]
        arr = np.asarray(arr, f32)
        base[row0:row0 + arr.shape[0], off:off + arr.shape[1]] = arr

    for d in range(2):
        put(f"wioux{d}", WiouX[d * P:(d + 1) * P])
        put(f"wfx{d}", np.asarray(inputs["W_fx"], f32)[d * P:(d + 1) * P])
    for k2 in range(2):
        put(f"wiouh{k2}", WiouH[k2 * P:(k2 + 1) * P])
        put(f"wfh{k2}", np.asarray(inputs["W_fh"], f32)[k2 * P:(k2 + 1) * P])
        put(f"wout{k2}", np.asarray(inputs["W_out"], f32)[k2 * P:(k2 + 1) * P])
    brow = np.zeros((1, cols["bias"][1]), f32)
    brow[0, :512] = bi512[0]
    brow[0, G3:G3 + H] = bf[0]
    brow[0, G3 + H:G3 + H + L] = np.asarray(inputs["b_out"], f32)
    put("bias", brow)
    put("ones", np.ones((1, P), f32))
    put("ident", np.eye(P, dtype=f32))
    emb_W = np.asarray(inputs["emb_W"], f32)
    rel_W = np.asarray(inputs["rel_W"], f32)

    ibase = np.zeros((P, max(CI, 1)), np.int32)

    in_maps = []
    for cd in per_core:
        bc = base.copy()
        for k in range(NKT):
            off, w = cols[f"GA{k}"]
            bc[:, off:off + cd["GA"].shape[2]] = cd["GA"][k]
            off, w = cols[f"Gp{k}"]
            bc[:, off:off + w] = cd["Gp"][k]
            off, w = cols[f"Pp{k}"]
            bc[:, off:off + w] = cd["Pperm"][k]
            off, w = cols[f"prow{k}"]
            bc[0, off:off + w] = cd["prow"][k]
        off, w = cols["Afc"]
        bc[:, off:off + cd["AfcL"].shape[1]] = cd["AfcL"]
        x = np.concatenate([emb_W[cd["xs_idx"][:, 0]],
                            rel_W[cd["rel_idx"][:, 0]]], axis=1)  # [Npad, 256]
        bc[:, cols["xT0"][0]:cols["xT0"][0] + x.shape[0]] = x[:, 0:P].T
        bc[:, cols["xT1"][0]:cols["xT1"][0] + x.shape[0]] = x[:, P:2 * P].T
        bi_ = ibase.copy()
        xo = icols["xsidx"][0]
        ro = icols["relidx"][0]
        mo = icols["masks"][0]
        for k in range(NKT):
            bi_[:, xo + k] = cd["xs_idx"][k * P:(k + 1) * P, 0]
            bi_[:, ro + k] = cd["rel_idx"][k * P:(k + 1) * P, 0]
        for m in range(NM):
            bi_[:, mo + m] = sizes["masks"][m][:, 0].astype(np.int32)
        in_maps.append(dict(
            bigc=np.ascontiguousarray(bc.astype(ml_dtypes.bfloat16)),
            bigi=np.ascontiguousarray(bi_),
        ))
    return in_maps


def kernel(**inputs):
    sizes, per_core = build_plan(inputs["xs"], inputs["rels"],
                                 inputs["child_idx"], inputs["parent_idx"],
                                 inputs["node_height"], int(inputs["n_levels"]))
    V, DE = np.asarray(inputs["emb_W"]).shape
    RV, DR = np.asarray(inputs["rel_W"]).shape
    L = np.asarray(inputs["W_out"]).shape[1]
    nc = build_bass(sizes, V, DE, RV, DR, L)
    in_maps = _make_in_maps(sizes, per_core, inputs)

    if os.environ.get("TREELSTM_SIM") == "1":
        from concourse.bass_interp import CoreSim
        ncores = int(os.environ.get("TREELSTM_SIM_CORES", N_CORES))
        outs = []
        for cid in range(ncores):
            sim = CoreSim(nc)
            for name, val in in_maps[cid].items():
                sim.tensor(name)[:] = val
            sim.simulate()
            outs.append(np.array(sim.tensor("out")).T)
        return np.concatenate(outs, axis=0).astype(np.float32)

    from concourse.bass_utils import run_bass_kernel_spmd
    res = run_bass_kernel_spmd(nc, in_maps, core_ids=list(range(N_CORES)),
                               trace=bool(int(os.environ.get("TREELSTM_TRACE", "0"))))
    if getattr(kernel, "_keep_results", False):
        kernel.last_results = res
    out = np.concatenate([r["out"].T for r in res.results], axis=0)
    return out.astype(np.float32)
